# revision 1
# baseline (speedup 1.0000x reference)
"""Trainium2 Bass kernel for LIFNet (leaky-integrator net, no spiking).

Math: the module is linear, and the leaky integration L (a causal LTI filter
along T) commutes with the per-timestep linear layers:

    V2 = L(L(batch @ W1^T) @ W2^T) = (L^2)(batch @ (W2 @ W1)^T)

with Wc = W2 @ W1 of shape [10, 784].  L^2 has impulse response
h[m] = beta^2 (m-1) alpha^(m-2) (m >= 2), which decays below f32 noise by
lag ~128, so the filter is applied as a banded blocked matmul with two
constant 128x128 blocks (intra-block R0, previous-block R1).

Device work per core (13 b's of [2000, 784], data-parallel over batch):
  - z^T = Wc @ x^T via PE matmuls (Wc chunks stationary, K=112 d-chunks,
    N=500 t-columns), accumulated in PSUM.
  - z^T [10, 128] slices -> z [128, 10] via PE transpose (tiny, cheap).
  - V2^T[o, t'-block] = R1^T-term + R0^T-term via two K=128 PE matmuls.
  - V2^T [10, 2000] DMA'd out per b; host re-assembles [100, 2000, 10].

Input is host-pretransposed to [cores, 13, 112, 7, 2000] so DMA loads put the
contraction dim (d) on partitions with 2 KB contiguous runs at full HBM BW.
"""

import sys

import numpy as np

for _p in ("/opt/trn_rl_repo",):
    if _p not in sys.path:
        sys.path.append(_p)

B, T, DIN, H1, H2 = 100, 2000, 784, 100, 10
ALPHA, BETA = 0.7, 0.3

NCORES = 8
BPAD = 104          # batch padded to 8 * 13
BP = BPAD // NCORES  # 13 b's per core
DC = 112            # d-chunk width (784 = 7 * 112), partition dim of x tiles
NDC = DIN // DC     # 7
TG = 500            # t-columns per z-matmul group (N <= 512)
NTG = T // TG       # 4
TB = 128            # t'-block for the filter stage
NTB = (T + TB - 1) // TB  # 16
TPADF = NTB * TB    # 2048 free-dim padding for the z^T staging buffer

_CACHE: dict = {}


def _filter_blocks() -> np.ndarray:
    """R = [R1 | R0] as [128, 256] f32: rhs blocks for the filter matmuls.

    out[o, t'] += sum_tl z_block[tl, o] * R[tl, t'] with R[tl, t'] =
    h[lag], lag = (t' - tl) + 128 for R1 (z from previous t-block) and
    (t' - tl) for R0 (intra-block, strictly causal).
    """
    m = np.arange(512, dtype=np.float64)
    h = np.zeros(512)
    h[2:] = BETA * BETA * (m[2:] - 1.0) * ALPHA ** (m[2:] - 2.0)
    tl = np.arange(TB)[:, None]
    tp = np.arange(TB)[None, :]
    r1 = h[tp - tl + TB]
    lag0 = tp - tl
    r0 = np.where(lag0 >= 2, h[np.clip(lag0, 0, None)], 0.0)
    return np.concatenate([r1, r0], axis=1).astype(np.float32)


def _build(reps: int = 1):
    """Build + compile the per-core Bass kernel (shared by all 8 cores).

    reps>1 repeats the whole compute body (for benchmarking device time
    independent of the PJRT/axon dispatch floor)."""
    from contextlib import ExitStack

    import concourse.tile as tile
    from concourse import bacc, mybir

    f32 = mybir.dt.float32
    f32r = mybir.dt.float32r
    nc = bacc.Bacc(
        "TRN2", target_bir_lowering=False, debug=False, num_devices=NCORES
    )

    xT = nc.dram_tensor("xT", [BP, DC, NDC, T], f32r, kind="ExternalInput")
    wct = nc.dram_tensor("wct", [DC, NDC * H2], f32r, kind="ExternalInput")
    rh = nc.dram_tensor("rh", [TB, 2 * TB], f32, kind="ExternalInput")
    eye = nc.dram_tensor("eye", [H2, H2], f32, kind="ExternalInput")
    vout = nc.dram_tensor("vout", [BP, H2, T], f32, kind="ExternalOutput")

    with tile.TileContext(nc) as tc, ExitStack() as ctx:
        const = ctx.enter_context(tc.tile_pool(name="const", bufs=1))
        xpool = ctx.enter_context(tc.tile_pool(name="xp", bufs=3))
        ring = ctx.enter_context(tc.tile_pool(name="ring", bufs=1))
        zbp = ctx.enter_context(tc.tile_pool(name="zbp", bufs=2))
        vsb = ctx.enter_context(tc.tile_pool(name="vsb", bufs=2))
        zpsum = ctx.enter_context(tc.tile_pool(name="zps", bufs=2, space="PSUM"))
        tpsum = ctx.enter_context(tc.tile_pool(name="tps", bufs=3, space="PSUM"))
        vpsum = ctx.enter_context(tc.tile_pool(name="vps", bufs=3, space="PSUM"))

        wct_sb = const.tile([DC, NDC * H2], f32r, tag="wct")
        nc.sync.dma_start(wct_sb[:], wct.ap())
        rh_sb = const.tile([TB, 2 * TB], f32, tag="rh")
        nc.sync.dma_start(rh_sb[:], rh.ap())
        eye_sb = const.tile([H2, H2], f32, tag="eye")
        nc.sync.dma_start(eye_sb[:], eye.ap())

        # Two-deep manual ring: the t-pad cols (>=2000) of the z^T staging
        # tile must stay zero across b iterations, so memset only once.
        zts_ring = []
        for i in range(2):
            zt = ring.tile([H2, TPADF], f32, tag=f"zts{i}", name=f"zts{i}")
            nc.vector.memset(zt[:], 0.0)
            zts_ring.append(zt)

        for rep in range(reps):
          for b in range(BP):
            zts = zts_ring[b % 2]

            # z^T[o, t] = sum_d Wc[o, d] x[t, d], d-chunks of 112 on partitions
            # One 3.1 MB DMA per half-b (8 KB-run descriptors), two 500-col
            # matmul groups sliced from each half tile.
            for h in range(2):
                xt = xpool.tile([DC, NDC * (T // 2)], f32r, tag="xt")
                nc.sync.dma_start(
                    xt[:].rearrange("p (c t) -> p c t", c=NDC),
                    xT.ap()[b, :, :, h * (T // 2) : (h + 1) * (T // 2)],
                )
                for gg in range(2):
                    g = 2 * h + gg
                    zp = zpsum.tile([H2, TG], f32, tag="zp")
                    for c in range(NDC):
                        nc.tensor.matmul(
                            zp[:],
                            wct_sb[:, c * H2 : (c + 1) * H2],
                            xt[:, c * (T // 2) + gg * TG : c * (T // 2) + gg * TG + TG],
                            start=(c == 0),
                            stop=(c == NDC - 1),
                        )
                    nc.scalar.copy(zts[0:H2, g * TG : (g + 1) * TG], zp[:])

            # z[t, o] per 128-t-block via PE transpose of z^T slices
            zb = zbp.tile([TB, NTB * H2], f32, tag="zb")
            for j in range(NTB):
                ztp = tpsum.tile([TB, H2], f32, tag="ztp")
                nc.tensor.transpose(
                    ztp[:], zts[0:H2, j * TB : (j + 1) * TB], eye_sb[:]
                )
                nc.scalar.copy(zb[:, j * H2 : (j + 1) * H2], ztp[:])

            # V2^T[o, 128-t'-block] = sum over prev/current z t-blocks
            v2 = vsb.tile([H2, T], f32, tag="v2")
            for j in range(NTB):
                vp = vpsum.tile([H2, TB], f32, tag="vp")
                n_mm = 2 if j > 0 else 1
                mm = 0
                for roff, jj in ((0, j - 1), (TB, j)):
                    if jj < 0:
                        continue
                    nc.tensor.matmul(
                        vp[:],
                        zb[:, jj * H2 : (jj + 1) * H2],
                        rh_sb[:, roff : roff + TB],
                        start=(mm == 0),
                        stop=(mm == n_mm - 1),
                    )
                    mm += 1
                w = min(TB, T - j * TB)
                nc.scalar.copy(v2[0:H2, j * TB : j * TB + w], vp[0:H2, 0:w])

            nc.sync.dma_start(vout.ap()[b], v2[:])

    nc.compile()
    return nc


def _prep_inputs(batch: np.ndarray, W1: np.ndarray, W2: np.ndarray):
    wc = (W2.astype(np.float64) @ W1.astype(np.float64)).astype(np.float32)
    # [112, 7*10]: wct[p, c*10+o] = Wc[o, 112c + p]
    wct = np.ascontiguousarray(
        wc.T.reshape(NDC, DC, H2).transpose(1, 0, 2).reshape(DC, NDC * H2)
    )
    rh = _filter_blocks()
    eye = np.eye(H2, dtype=np.float32)

    bp = np.zeros((BPAD, T, DIN), np.float32)
    bp[:B] = batch
    # [8, 13, 112, 7, 2000]: core, b, d%112 (partitions), d-chunk, t
    xt = np.ascontiguousarray(
        bp.reshape(NCORES, BP, T, NDC, DC).transpose(0, 1, 4, 3, 2)
    )
    return xt, wct, rh, eye


def kernel(batch: np.ndarray, W1: np.ndarray, W2: np.ndarray) -> np.ndarray:
    from concourse import bass_utils

    if "nc" not in _CACHE:
        _CACHE["nc"] = _build()
    nc = _CACHE["nc"]

    xt, wct, rh, eye = _prep_inputs(batch, W1, W2)
    in_maps = [
        {"xT": xt[i], "wct": wct, "rh": rh, "eye": eye} for i in range(NCORES)
    ]
    res = bass_utils.run_bass_kernel_spmd(
        nc, in_maps, core_ids=list(range(NCORES)), **_CACHE.get("run_kwargs", {})
    )
    _CACHE["last_result"] = res

    full = np.concatenate([r["vout"] for r in res.results], axis=0)  # [104,10,2000]
    return np.ascontiguousarray(full.transpose(0, 2, 1)[:B])



# revision 2
# speedup vs baseline: 1.9308x; 1.9308x over previous
"""Trainium2 Bass kernel for LIFNet (leaky-integrator net, no spiking).

Math: the module is linear, and the leaky integration L (a causal LTI filter
along T) commutes with the per-timestep linear layers:

    V2 = L(L(batch @ W1^T) @ W2^T) = (L^2)(batch @ (W2 @ W1)^T)

with Wc = W2 @ W1 of shape [10, 784].  L^2 has impulse response
h[m] = beta^2 (m-1) alpha^(m-2) (m >= 2), which decays below f32 noise by
lag ~128, so the filter is applied as a banded blocked matmul with two
constant 128x128 blocks (intra-block R0, previous-block R1).

The kernel is HBM-bandwidth-bound (the batch read dominates), so:
  - x is pre-cast to fp16 on the host (quantization adds ~3e-4 rel err
    against a 2e-2 gate) halving DMA bytes.
  - The DRAM layout is per-partition contiguous per b ([112, 14000] runs
    of 28 KB), so each DMA is 112 x 28KB descriptors instead of the
    4 KB descriptors that throttled the f32 version to ~207 GB/s.

Device work per core (13 b's, data-parallel over batch; groups of 4 b's
packed 32-partitions apart so downstream stages run 4 b's per instruction):
  - z^T = Wc @ x^T via PE matmuls (fp16, Wc chunks [112, 32] zero-padded,
    tile_position=(0, 32i) places b_i's output rows at psum partition 32i).
  - zp [128, 500] f32 -> zts [128, 2048] fp16 (cast copy, scalar engine).
  - PE transpose per 128-t-block: [128, 128] -> tpsum fp16; DVE compacts
    the 4x10 used columns into zb slabs [128, 40].
  - V2^T[4 b's] per t'-block via two K=128 fp16 matmuls (R1 prev / R0 cur).
  - v2 [40, 2000] f32 DMA'd out per group; host re-assembles.
"""

import sys

import numpy as np

for _p in ("/opt/trn_rl_repo",):
    if _p not in sys.path:
        sys.path.append(_p)

B, T, DIN, H1, H2 = 100, 2000, 784, 100, 10
ALPHA, BETA = 0.7, 0.3

NCORES = 8
BPAD = 104           # batch padded to 8 * 13
BP = BPAD // NCORES  # 13 b's per core
DC = 112             # d-chunk width (784 = 7 * 112), partition dim of x tiles
NDC = DIN // DC      # 7
MP = 32              # padded output rows per b (10 real + 22 zero)
TG = 500             # t-columns per z-matmul group (one psum bank)
NTG = T // TG        # 4
TB = 128             # t'-block for the filter stage
NTB = (T + TB - 1) // TB   # 16
TPADF = NTB * TB     # 2048 free-dim padding for the z^T staging buffer
GROUPS = [(0, 4), (4, 4), (8, 4), (12, 1)]  # (first b, group size)

_CACHE: dict = {}


def _filter_blocks() -> np.ndarray:
    """R = [R1 | R0] as [128, 256] fp16: rhs blocks for the filter matmuls.

    out[o, t'] += sum_tl z_block[tl, o] * R[tl, t'] with R[tl, t'] =
    h[lag], lag = (t' - tl) + 128 for R1 (z from previous t-block) and
    (t' - tl) for R0 (intra-block, strictly causal).
    """
    m = np.arange(512, dtype=np.float64)
    h = np.zeros(512)
    h[2:] = BETA * BETA * (m[2:] - 1.0) * ALPHA ** (m[2:] - 2.0)
    tl = np.arange(TB)[:, None]
    tp = np.arange(TB)[None, :]
    r1 = h[tp - tl + TB]
    lag0 = tp - tl
    r0 = np.where(lag0 >= 2, h[np.clip(lag0, 0, None)], 0.0)
    return np.concatenate([r1, r0], axis=1).astype(np.float16)


def _build(reps: int = 1):
    """Build + compile the per-core Bass kernel (shared by all 8 cores)."""
    from contextlib import ExitStack

    import concourse.tile as tile
    from concourse import bacc, mybir

    f16 = mybir.dt.float16
    f32 = mybir.dt.float32
    nc = bacc.Bacc(
        "TRN2", target_bir_lowering=False, debug=False, num_devices=NCORES
    )

    xT = nc.dram_tensor("xT", [BP, DC, NDC * T], f16, kind="ExternalInput")
    wct = nc.dram_tensor("wct", [DC, NDC * MP], f16, kind="ExternalInput")
    rh = nc.dram_tensor("rh", [TB, 2 * TB], f16, kind="ExternalInput")
    eye = nc.dram_tensor("eye", [TB, TB], f16, kind="ExternalInput")
    vout = nc.dram_tensor("vout", [BP * H2, T], f32, kind="ExternalOutput")

    with tile.TileContext(nc) as tc, ExitStack() as ctx:
        const = ctx.enter_context(tc.tile_pool(name="const", bufs=1))
        xpool = ctx.enter_context(tc.tile_pool(name="xp", bufs=3))
        ring = ctx.enter_context(tc.tile_pool(name="ring", bufs=1))
        zbp = ctx.enter_context(tc.tile_pool(name="zbp", bufs=2))
        vsb = ctx.enter_context(tc.tile_pool(name="vsb", bufs=2))
        zpsum = ctx.enter_context(tc.tile_pool(name="zps", bufs=1, space="PSUM"))
        tpsum = ctx.enter_context(tc.tile_pool(name="tps", bufs=2, space="PSUM"))
        vpsum = ctx.enter_context(tc.tile_pool(name="vps", bufs=2, space="PSUM"))

        # consts on the scalar HWDGE queue so they don't delay the first
        # x load on the sync queue
        wct_sb = const.tile([DC, NDC * MP], f16, tag="wct")
        nc.scalar.dma_start(wct_sb[:], wct.ap())
        rh_sb = const.tile([TB, 2 * TB], f16, tag="rh")
        nc.scalar.dma_start(rh_sb[:], rh.ap())
        eye_sb = const.tile([TB, TB], f16, tag="eye")
        nc.scalar.dma_start(eye_sb[:], eye.ap())

        # Two-deep manual ring: the t-pad cols (>=2000) of the z^T staging
        # tile must stay zero across groups, so memset only once.
        zts_ring = []
        for i in range(2):
            zt = ring.tile([TB, TPADF], f16, tag=f"zts{i}", name=f"zts{i}")
            nc.vector.memset(zt[:], 0.0)
            zts_ring.append(zt)

        for rep in range(reps):
          for g, (b0, G) in enumerate(GROUPS):
            zts = zts_ring[g % 2]
            PG = MP * G

            # z^T[o, t] = sum_d Wc[o, d] x[t, d]; b_i's rows at psum
            # partitions 32i via PE column tiling.
            zp_tiles = [
                zpsum.tile([TB, TG], f32, tag=f"zp{tg}", name=f"zp{tg}")
                for tg in range(NTG)
            ]
            for i in range(G):
                b = b0 + i
                xt = xpool.tile([DC, NDC * T], f16, tag="xt")
                nc.sync.dma_start(xt[:], xT.ap()[b])
                xv = xt[:].rearrange("p (c t) -> p c t", c=NDC)
                for tg in range(NTG):
                    zp = zp_tiles[tg]
                    for c in range(NDC):
                        nc.tensor.matmul(
                            zp[MP * i : MP * (i + 1), :],
                            wct_sb[:, c * MP : (c + 1) * MP],
                            xv[:, c, tg * TG : (tg + 1) * TG],
                            start=(c == 0),
                            stop=(c == NDC - 1),
                            tile_position=(0, MP * i),
                        )
            for tg in range(NTG):
                nc.scalar.copy(
                    zts[0:PG, tg * TG : (tg + 1) * TG], zp_tiles[tg][0:PG, :]
                )

            # z[t, o] per 128-t-block via PE transpose; DVE compacts the
            # 4x10 used columns into dense 40-wide slabs of zb.
            zb = zbp.tile([TB, NTB * 4 * H2], f16, tag="zb")
            zbv = zb[:].rearrange("p (j gg o) -> p j gg o", j=NTB, gg=4)
            for j in range(NTB):
                tp = tpsum.tile([TB, TB], f16, tag="tp")
                nc.tensor.transpose(
                    tp[:], zts[:, j * TB : (j + 1) * TB], eye_sb[:]
                )
                tpv = tp[:].rearrange("p (gg o) -> p gg o", gg=4)
                nc.vector.tensor_copy(zbv[:, j, 0:G, :], tpv[:, 0:G, 0:H2])

            # V2^T[4 b's, 128-t'-block] = R1-term (prev block) + R0-term
            v2 = vsb.tile([4 * H2, T], f32, tag="v2")
            OG = H2 * G
            for j in range(NTB):
                vp = vpsum.tile([4 * H2, TB], f32, tag="vp")
                n_mm = 2 if j > 0 else 1
                mm = 0
                for roff, jj in ((0, j - 1), (TB, j)):
                    if jj < 0:
                        continue
                    nc.tensor.matmul(
                        vp[0:OG, :],
                        zb[:, jj * 4 * H2 : jj * 4 * H2 + OG],
                        rh_sb[:, roff : roff + TB],
                        start=(mm == 0),
                        stop=(mm == n_mm - 1),
                    )
                    mm += 1
                w = min(TB, T - j * TB)
                nc.scalar.copy(v2[0:OG, j * TB : j * TB + w], vp[0:OG, 0:w])

            # outputs on the scalar HWDGE queue (parallel to x loads)
            nc.scalar.dma_start(
                vout.ap()[H2 * b0 : H2 * b0 + OG, :], v2[0:OG, :]
            )

    nc.compile()
    return nc


def _prep_inputs(batch: np.ndarray, W1: np.ndarray, W2: np.ndarray):
    wc = (W2.astype(np.float64) @ W1.astype(np.float64)).astype(np.float32)
    # [112, 7, 32]: wct[p, c, o] = Wc[o, 112c + p] for o < 10, else 0
    wct = np.zeros((DC, NDC, MP), np.float16)
    wct[:, :, :H2] = wc.T.reshape(NDC, DC, H2).transpose(1, 0, 2)
    wct = np.ascontiguousarray(wct.reshape(DC, NDC * MP))
    rh = _filter_blocks()
    eye = np.eye(TB, dtype=np.float16)

    bp = np.zeros((BPAD, T, DIN), np.float16)
    bp[:B] = batch.astype(np.float16)
    # [8, 13, 112, 7*2000]: per-partition runs of 28 KB
    xt = np.ascontiguousarray(
        bp.reshape(NCORES, BP, T, NDC, DC).transpose(0, 1, 4, 3, 2)
    ).reshape(NCORES, BP, DC, NDC * T)
    return xt, wct, rh, eye


def kernel(batch: np.ndarray, W1: np.ndarray, W2: np.ndarray) -> np.ndarray:
    from concourse import bass_utils

    if "nc" not in _CACHE:
        _CACHE["nc"] = _build()
    nc = _CACHE["nc"]

    xt, wct, rh, eye = _prep_inputs(batch, W1, W2)
    in_maps = [
        {"xT": xt[i], "wct": wct, "rh": rh, "eye": eye} for i in range(NCORES)
    ]
    res = bass_utils.run_bass_kernel_spmd(
        nc, in_maps, core_ids=list(range(NCORES)), **_CACHE.get("run_kwargs", {})
    )
    _CACHE["last_result"] = res

    full = np.concatenate(
        [r["vout"].reshape(BP, H2, T) for r in res.results], axis=0
    )  # [104, 10, 2000]
    return np.ascontiguousarray(full.transpose(0, 2, 1)[:B])
